# revision 3
# baseline (speedup 1.0000x reference)
"""Trainium2 Bass kernel for nn_MeanAggregator:

    out = features[nodes] + mean(features[neigh_idx], axis=1)

Sharding: batch data-parallel across 8 NeuronCores (12500 nodes/core,
padded to 12544 = 98 tiles of 128), feature table replicated per core.
Per 128-node tile, 17 SWDGE indirect DMAs each gather 128 rows (one
offset per partition; HW streams the full contiguous dest extent per
partition, so one instruction = exactly 128 descriptors) into SBUF
[128, 17*128]. DVE tree-adds the 16 neighbor blocks, then a fused
scalar_tensor_tensor computes (sum * 1/16) + self. HWDGE writes the
[128,128] result back to DRAM. No cross-core communication.

This is at the structural floor of the hardware for this workload.
Trace-verified facts (see session probes) for future optimization
attempts:
- Pool/Q7 engine busy is 1126ns per indirect DMA + a constant 310ns
  sequencer handoff gap (p50 == p90 == 310ns, zero stalls): 1666
  instructions x 1436ns = 2.39ms. Pool buffer depths (gbufs=8,
  abufs=4, obufs=6) are the measured optimum.
- ALL dynamic descriptor generation serializes on the single Q7 SWDGE
  at ~9-11ns/descriptor+overhead; random 512B rows need one descriptor
  each, so 212.5k rows/core => ~2.4ms regardless of primitive:
  - indirect_dma_start: 128 rows / 1.44us = 11.2ns/row (this kernel).
  - InstDMAGatherAnt (int16 idx, 32k-row window, needs
    single_packet=False above ~1k idxs, <= ~4k idxs/instr ring cap):
    8.7ns/row end-to-end in a clean pure-gather pipeline (7.2ns/row Q7
    busy, ~46ns gaps, Pool 91% busy, DMA engines idle at ~59GB/s).
    Best possible future kernel = window-sorted dma_gather (1.96ms for
    225k rows) + off-Q7 PE-selection combine: ~2.0ms ceiling (~1.2x).
  - InstDMAScatterAddAnt (SBUF parity dest): 16.6us per 1920 tokens;
    also loses updates on same-dest tokens co-resident in the DMA ring
    (probabilistic CCE RMW race; distance does not fix it).
- num_swdge_queues=4 round-robin does NOT parallelize Q7 desc-gen
  (measured 2.60ms, a regression).
- Multi-offset-per-partition indirect DMA (strided dest) is broken on
  HW (garbage offsets / DMA abort); the sim's per-offset semantics do
  not match hardware.
- The cost model's SWDGE_NS_PER_DESCRIPTOR=0.34 is wrong for custom
  SWDGE ops; trust only HW traces.
"""

import numpy as np

import concourse.bass as bass
import concourse.mybir as mybir
import concourse.tile as tile
from concourse import bacc
from concourse.bass_utils import run_bass_kernel_spmd

NUM_NODES = 1_000_000
FEAT = 128
BATCH = 100_000
K = 16
BLK = K + 1  # rows gathered per node: self + K neighbors
CORES = 8
B_CORE = BATCH // CORES  # 12500
P = 128
N_TILES = (B_CORE + P - 1) // P  # 98
B_PAD = N_TILES * P  # 12544

_NC_CACHE: dict = {}


def build_nc(n_tiles=N_TILES, table_rows=NUM_NODES, gbufs=8, abufs=4, obufs=6):
    f32 = mybir.dt.float32
    i32 = mybir.dt.int32
    nc = bacc.Bacc(None, debug=False)
    feat_t = nc.dram_tensor("features", [table_rows, FEAT], f32, kind="ExternalInput")
    idx_t = nc.dram_tensor("idx", [P, n_tiles * BLK], i32, kind="ExternalInput")
    out_t = nc.dram_tensor("out", [n_tiles, P, FEAT], f32, kind="ExternalOutput")

    with tile.TileContext(nc) as tc:
        with (
            tc.tile_pool(name="idxp", bufs=1) as idxp,
            tc.tile_pool(name="gp", bufs=gbufs) as gp,
            tc.tile_pool(name="tp", bufs=abufs) as tp,
            tc.tile_pool(name="op", bufs=obufs) as op,
        ):
            # HW indirect-DMA semantics (probed): ONE offset per partition,
            # streaming that partition's full contiguous dest extent from
            # features[offset]. So each instruction gathers 128 rows (one
            # per partition); a tile of 128 nodes x 17 rows needs 17
            # instructions, one per slot t, writing block t of G.
            idx_sb = idxp.tile([P, n_tiles * BLK], i32)
            nc.sync.dma_start(out=idx_sb[:], in_=idx_t[:])
            for n in range(n_tiles):
                # G[p, t, 0:128] = features[idx[p, n*17+t]]
                G = gp.tile([P, BLK, FEAT], f32)
                for t in range(BLK):
                    nc.gpsimd.indirect_dma_start(
                        out=G[:, t, :],
                        out_offset=None,
                        in_=feat_t[:],
                        in_offset=bass.IndirectOffsetOnAxis(
                            ap=idx_sb[:, n * BLK + t : n * BLK + t + 1], axis=0
                        ),
                    )
                # tree-add the 16 neighbor blocks (t=1..16)
                A = tp.tile([P, 15, FEAT], f32)
                nc.vector.tensor_add(A[:, 0:8, :], G[:, 1:9, :], G[:, 9:17, :])
                nc.vector.tensor_add(A[:, 8:12, :], A[:, 0:4, :], A[:, 4:8, :])
                nc.vector.tensor_add(A[:, 12:14, :], A[:, 8:10, :], A[:, 10:12, :])
                nc.vector.tensor_add(A[:, 14:15, :], A[:, 12:13, :], A[:, 13:14, :])
                O = op.tile([P, FEAT], f32)
                nc.vector.scalar_tensor_tensor(
                    out=O[:],
                    in0=A[:, 14, :],
                    scalar=1.0 / K,
                    in1=G[:, 0, :],
                    op0=mybir.AluOpType.mult,
                    op1=mybir.AluOpType.add,
                )
                nc.sync.dma_start(out=out_t[n], in_=O[:])
    nc.finalize()
    return nc


def _get_nc():
    if "nc" not in _NC_CACHE:
        _NC_CACHE["nc"] = build_nc()
    return _NC_CACHE["nc"]


def _shard_idx(idx_all):
    """idx_all [BATCH, BLK] int32 -> per-core [P, N_TILES*BLK] tiles-major layout."""
    maps = []
    for c in range(CORES):
        sh = idx_all[c * B_CORE : (c + 1) * B_CORE]
        pad = np.zeros((B_PAD, BLK), np.int32)
        pad[:B_CORE] = sh
        r = pad.reshape(N_TILES, P, BLK).transpose(1, 0, 2).reshape(P, N_TILES * BLK)
        maps.append(np.ascontiguousarray(r))
    return maps


def run_sharded(features, nodes, neigh_idx, trace=False, **spmd_kwargs):
    features = np.ascontiguousarray(np.asarray(features, dtype=np.float32))
    nodes = np.asarray(nodes).astype(np.int32)
    neigh_idx = np.asarray(neigh_idx).astype(np.int32)
    idx_all = np.concatenate([nodes[:, None], neigh_idx], axis=1)
    in_maps = [
        {"features": features, "idx": idx_c} for idx_c in _shard_idx(idx_all)
    ]
    res = run_bass_kernel_spmd(
        _get_nc(), in_maps, list(range(CORES)), trace=trace, **spmd_kwargs
    )
    out = np.concatenate(
        [res.results[c]["out"].reshape(B_PAD, FEAT)[:B_CORE] for c in range(CORES)],
        axis=0,
    )
    return out, res


def kernel(**inputs):
    num_sample = int(np.asarray(inputs["num_sample"]))
    assert num_sample == K, f"kernel hardcodes K={K}, got {num_sample}"
    out, _ = run_sharded(
        inputs["features"], inputs["nodes"], inputs["neigh_idx"], trace=False
    )
    return out

